# revision 6
# baseline (speedup 1.0000x reference)
"""Trainium2 Bass kernel for nn_MixedSparseGatedMLP (LoRA-augmented gated MLP).

Math (reference):
    y1 = x @ Wg + (x @ Ag) @ Bg
    y2 = x @ Wu + (x @ Au) @ Bu
    x3 = relu(y1) * y2
    y3 = x3 @ Wd + (x3 @ Ad) @ Bd

Strategy:
  - Fold the rank-16 LoRA factors into the dense weights on the host
    (exact fp32 algebra): Wg_eff = Wg + Ag@Bg, etc.  The device kernel is
    then a plain gated MLP with three dense matmuls.
  - TP2 x DP4 sharding: I = 11008 = 86*128 chunks split into 2
    tensor-parallel groups of 43 chunks (no padding, unlike an 8-way
    split which needs 88), and tokens split 4 ways (1024 per core, as
    2 blocks of 512).  Core c: group g=c//4, token range d=c%4.
    Each core emits a partial out^T [H, 1024]; the host adds the two
    group partials per token range (gather/unshard step).
  - Down projection runs in out^T orientation: stationary = Wd slice
    [128i x 128h], moving = x3 [128i x 512t], one PSUM tile per
    128-row h-block, 43-deep accumulation.  Wd streams through SBUF
    exactly once per block (it is far too big to keep resident).
  - bf16 operands, fp32 PSUM accumulation, fp32 partial outputs.
  - All DRAM layouts are pre-tiled on the host so every DMA is a linear
    copy into the exact SBUF layout the matmuls need.
"""

import os
import sys

for _p in ("/opt/trn_rl_repo", "/root/.axon_site/_ro/trn_rl_repo"):
    if os.path.isdir(_p) and _p not in sys.path:
        sys.path.append(_p)

import numpy as np
import ml_dtypes

# Problem shapes (hardcoded per contract)
B, S, H, I, R = 2, 2048, 4096, 11008, 16
NTOK = B * S              # 4096 tokens
NCORES = 8
TPG = 2                   # tensor-parallel groups over I
DPG = 4                   # data-parallel groups over tokens
CI = I // 128             # 86 i-chunks total
C = CI // TPG             # 43 i-chunks per core
IS = C * 128              # 5504 intermediate columns per core
K = H // 128              # 32 h-chunks (gate/up contraction)
TB = 512                  # token block
TOKC = NTOK // DPG        # 1024 tokens per core
NB = TOKC // TB           # 2 token blocks per core
NHB = H // 128            # 32 output h-blocks (down phase)
KG = 8                    # x k-groups per block (startup split)
KS = K // KG

BF16 = ml_dtypes.bfloat16

# set by test.py for profiling; harness path leaves these as-is
TRACE = False
LAST_EXEC_TIME_NS = None
LAST_RESULTS = None


def _build_nc():
    import concourse.bacc as bacc
    import concourse.mybir as mybir
    import concourse.tile as tile

    bf16 = mybir.dt.bfloat16
    f32 = mybir.dt.float32

    nc = bacc.Bacc("TRN2", target_bir_lowering=False, debug=False)

    # DRAM parameters (host pre-tiled layouts; see kernel() for the math)
    x = nc.declare_dram_parameter("x", [NB, 128, K * TB], bf16, isOutput=False)
    wg = nc.declare_dram_parameter("wg", [C, 128, K * 128], bf16, isOutput=False)
    wu = nc.declare_dram_parameter("wu", [C, 128, K * 128], bf16, isOutput=False)
    wd = nc.declare_dram_parameter("wd", [NHB, 128, C * 128], bf16, isOutput=False)
    out = nc.declare_dram_parameter("out", [H, TOKC], f32, isOutput=True)

    with tile.TileContext(nc) as tc:
        with tc.tile_pool(name="xp", bufs=1) as xp, \
             tc.tile_pool(name="wp", bufs=4) as wp, \
             tc.tile_pool(name="wdp", bufs=3) as wdp, \
             tc.tile_pool(name="x3p", bufs=1) as x3p, \
             tc.tile_pool(name="rp", bufs=2) as rp, \
             tc.tile_pool(name="op", bufs=4) as op, \
             tc.tile_pool(name="pgp", bufs=3, space="PSUM") as pgp, \
             tc.tile_pool(name="pup", bufs=3, space="PSUM") as pup, \
             tc.tile_pool(name="pdp", bufs=2, space="PSUM") as pdp:

            # PE warmup: dependency-free matmuls run during the initial DMA
            # wait and lift the HAM clock gate to 8/8 before real work.
            # Count sized to end when the first wg/x tiles land (~12us).
            warm_in = rp.tile([128, TB], bf16, tag="warm")
            nc.any.memset(warm_in, 0.0)
            warm_ps = pdp.tile([128, TB], f32, tag="pd")
            for _ in range(12):
                nc.tensor.matmul(warm_ps, warm_in[:, 0:128], warm_in,
                                 start=True, stop=True)

            def w_tile(src, m, eng=None):
                # monolithic 1MB weight DMAs: large transfers fan out over
                # more DMA engines and sustain ~2x the per-queue bandwidth
                # of 256KB tiles
                t = wp.tile([128, K * 128], bf16, tag="w")
                (eng or nc.sync).dma_start(t, src[m])
                return t

            def wd_tile(n, eng=None):
                t = wdp.tile([128, C * 128], bf16, tag="wdn")
                (eng or nc.sync).dma_start(t, wd[n])
                return t

            # prefetched tiles: (b, m) -> (wg_tile, wu_tile); (b, n) -> wd tile
            wpref = {}
            wdpref = {}
            xbg_next = {}

            for b in range(NB):
                # x block, split into KG tiles: [128 h-in-chunk, (k, t)] bf16
                xbg = []
                if b == 0:
                    # Startup ramp is HBM-bound with no previous block to
                    # hide under.  Interleave the ~6MB critical mass across
                    # both DGE paths in need-time order (the gate k-loop
                    # consumes x groups sequentially at ~0.85us each):
                    #   HWDGE: wg0, x1, x3, x5, x7  (+wg1... stream after)
                    #   SWDGE: x0, x2, x4, x6, wu0
                    xbg = [None] * KG
                    def xdma(gi, eng):
                        t = xp.tile([128, KS * TB], bf16, tag=f"xb{gi}")
                        eng.dma_start(t, x[0][:, gi * KS * TB:(gi + 1) * KS * TB])
                        xbg[gi] = t
                    xdma(0, nc.gpsimd)
                    wgt0 = w_tile(wg, 0)
                    xdma(1, nc.sync)
                    xdma(2, nc.gpsimd)
                    xdma(3, nc.sync)
                    xdma(4, nc.gpsimd)
                    xdma(5, nc.sync)
                    xdma(6, nc.gpsimd)
                    xdma(7, nc.sync)
                    wut0 = w_tile(wu, 0, nc.gpsimd)
                    wpref[(0, 0)] = (wgt0, wut0)
                else:
                    # prefetched during the previous down phase (SWDGE)
                    xbg = [xbg_next.pop(gi) for gi in range(KG)]

                # x3^T for this block: [128 i-in-chunk, (c, t)] bf16
                x3 = x3p.tile([128, C * TB], bf16, tag="x3")

                # ---- gate / up projections + gating, per i-chunk m ----
                for m in range(C):
                    if (b, m) in wpref:
                        wgt, wut = wpref.pop((b, m))
                    else:
                        # block 0: split the weight stream across both DGE
                        # paths (no down phase to hide under); later blocks
                        # stream both on HWDGE (146 GB/s, proven) while
                        # SWDGE stays free for prefetches.
                        wgt = w_tile(wg, m)
                        wut = w_tile(wu, m, nc.gpsimd if b == 0 else None)

                    g = pgp.tile([128, TB], f32, tag="pg")
                    u = pup.tile([128, TB], f32, tag="pu")
                    for k in range(K):
                        gi, kk = divmod(k, KS)
                        nc.tensor.matmul(
                            g,
                            wgt[:, k * 128:(k + 1) * 128],
                            xbg[gi][:, kk * TB:(kk + 1) * TB],
                            start=(k == 0), stop=(k == K - 1),
                        )
                    for k in range(K):
                        gi, kk = divmod(k, KS)
                        nc.tensor.matmul(
                            u,
                            wut[:, k * 128:(k + 1) * 128],
                            xbg[gi][:, kk * TB:(kk + 1) * TB],
                            start=(k == 0), stop=(k == K - 1),
                        )
                    # x3 = relu(g) * u ; DVE may read only one PSUM input,
                    # so relu lands in SBUF via ACT first.
                    r = rp.tile([128, TB], bf16, tag="r")
                    nc.scalar.activation(r, g, mybir.ActivationFunctionType.Relu)
                    nc.vector.tensor_mul(x3[:, m * TB:(m + 1) * TB], r, u)

                    if b == 0:
                        # wd prefetch for block 0's down phase: both DGE
                        # queues are lockstepped with the weight stream by
                        # wp backpressure, so insert the first wd tiles into
                        # the streams near the tail — they land ~2 chunks
                        # (~27us) before the down phase needs them.
                        if m == C - 3:
                            wdpref[(0, 0)] = wd_tile(0, nc.sync)
                        elif m == C - 2:
                            wdpref[(0, 1)] = wd_tile(1, nc.gpsimd)
                        elif m == C - 1:
                            wdpref[(0, 2)] = wd_tile(2, nc.sync)
                    elif m == 0:
                        # SWDGE is idle during later gate phases: pull the
                        # whole wd prefetch window up front.
                        for n in range(3):
                            wdpref[(b, n)] = wd_tile(n, nc.gpsimd)

                # ---- down projection, out^T orientation ----
                # psum [128 h, TB tok] accumulates over all 43 i-chunks;
                # wd streams h-block by h-block on HWDGE (JIT via wdp
                # backpressure), stores ride SWDGE.
                for n in range(NHB):
                    wdt = wdpref.pop((b, n), None)
                    if wdt is None:
                        wdt = wd_tile(n)
                    d = pdp.tile([128, TB], f32, tag="pd")
                    for c in range(C):
                        nc.tensor.matmul(
                            d,
                            wdt[:, c * 128:(c + 1) * 128],
                            x3[:, c * TB:(c + 1) * TB],
                            start=(c == 0), stop=(c == C - 1),
                        )
                    o = op.tile([128, TB], f32, tag="o")
                    nc.scalar.copy(o, d)
                    # last block: HWDGE's lower latency trims the kernel
                    # tail (SWDGE's DIRECT2D trigger costs ~5us extra)
                    st = nc.sync if b == NB - 1 else nc.gpsimd
                    st.dma_start(
                        out[n * 128:(n + 1) * 128, b * TB:(b + 1) * TB], o
                    )

                    if b < NB - 1:
                        # prefetch the next block's x and first gate/up
                        # weights on SWDGE while HWDGE streams wd
                        if 2 <= n < 2 + KG:
                            gi = n - 2
                            t = xp.tile([128, KS * TB], bf16, tag=f"xb{gi}")
                            nc.gpsimd.dma_start(
                                t, x[b + 1][:, gi * KS * TB:(gi + 1) * KS * TB]
                            )
                            xbg_next[gi] = t
                        elif n == 26:
                            wgt1 = w_tile(wg, 0, nc.gpsimd)
                        elif n == 28:
                            wut1 = w_tile(wu, 0, nc.gpsimd)
                            wpref[(b + 1, 0)] = (wgt1, wut1)

    nc.compile()
    return nc


def _prep_inputs(x1, w_gate, w_gate_lora_a, w_gate_lora_b,
                 w_up, w_up_lora_a, w_up_lora_b,
                 w_down, w_down_lora_a, w_down_lora_b):
    """Fold LoRA, shard TP2xDP4, and pre-tile DRAM layouts."""
    f32 = np.float32
    x1 = np.asarray(x1, f32)
    wg_eff = np.asarray(w_gate, f32) + np.asarray(w_gate_lora_a, f32) @ np.asarray(w_gate_lora_b, f32)
    wu_eff = np.asarray(w_up, f32) + np.asarray(w_up_lora_a, f32) @ np.asarray(w_up_lora_b, f32)
    wd_eff = np.asarray(w_down, f32) + np.asarray(w_down_lora_a, f32) @ np.asarray(w_down_lora_b, f32)

    x2d = x1.reshape(NTOK, H)

    # per-group weight tilings
    gmaps = []
    for g in range(TPG):
        sl = slice(g * IS, (g + 1) * IS)
        # wg tile layout: [m, p, k*128+j] = wg_eff[k*128+p, g*IS + m*128 + j]
        wgc = np.ascontiguousarray(
            wg_eff[:, sl].reshape(K, 128, C, 128).transpose(2, 1, 0, 3)
        ).astype(BF16).reshape(C, 128, K * 128)
        wuc = np.ascontiguousarray(
            wu_eff[:, sl].reshape(K, 128, C, 128).transpose(2, 1, 0, 3)
        ).astype(BF16).reshape(C, 128, K * 128)
        # wd tile layout: [n, p, c*128+h] = wd_eff[g*IS + c*128 + p, n*128+h]
        wdc = np.ascontiguousarray(
            wd_eff[sl, :].reshape(C, 128, NHB, 128).transpose(2, 1, 0, 3)
        ).astype(BF16).reshape(NHB, 128, C * 128)
        gmaps.append((wgc, wuc, wdc))

    # per-DP-slice x tilings: x_t[b, p, k*TB+t] = x2d[d*TOKC + b*TB + t, k*128+p]
    xmaps = []
    for d in range(DPG):
        xs = x2d[d * TOKC:(d + 1) * TOKC]
        xt = np.ascontiguousarray(
            xs.reshape(NB, TB, K, 128).transpose(0, 3, 2, 1)
        ).astype(BF16).reshape(NB, 128, K * TB)
        xmaps.append(xt)

    in_maps = []
    for ci in range(NCORES):
        g, d = divmod(ci, DPG)
        wgc, wuc, wdc = gmaps[g]
        in_maps.append({"x": xmaps[d], "wg": wgc, "wu": wuc, "wd": wdc})
    return in_maps


def _emulate(in_maps):
    """Numpy emulation of the device math (bf16 operands, fp32 accum),
    reconstructing operands from the tiled layouts to validate them."""
    f32 = np.float32
    acc = np.zeros((NTOK, H), f32)
    for ci, m in enumerate(in_maps):
        g, d = divmod(ci, DPG)
        xt = m["x"].reshape(NB, 128, K, TB)
        xs = xt.transpose(0, 3, 2, 1).reshape(TOKC, H).astype(f32)
        wgc = m["wg"].reshape(C, 128, K, 128)
        wg2 = wgc.transpose(2, 1, 0, 3).reshape(H, IS).astype(f32)
        wuc = m["wu"].reshape(C, 128, K, 128)
        wu2 = wuc.transpose(2, 1, 0, 3).reshape(H, IS).astype(f32)
        wdc = m["wd"].reshape(NHB, 128, C, 128)
        wd2 = wdc.transpose(2, 1, 0, 3).reshape(IS, H).astype(f32)
        y1 = xs @ wg2
        y2 = xs @ wu2
        r = np.maximum(y1, 0).astype(BF16).astype(f32)
        x3 = (r * y2).astype(BF16).astype(f32)
        acc[d * TOKC:(d + 1) * TOKC] += x3 @ wd2
    return acc.reshape(B, S, H)


def kernel(**inputs):
    global LAST_EXEC_TIME_NS, LAST_RESULTS
    in_maps = _prep_inputs(**inputs)

    if os.environ.get("KERNEL_EMULATE"):
        return _emulate(in_maps)

    from concourse.bass_utils import run_bass_kernel_spmd

    nc = _build_nc()
    res = run_bass_kernel_spmd(nc, in_maps, list(range(NCORES)), trace=TRACE)
    LAST_EXEC_TIME_NS = res.exec_time_ns
    LAST_RESULTS = res

    acc = np.zeros((NTOK, H), np.float32)
    for ci, r in enumerate(res.results):
        g, d = divmod(ci, DPG)
        acc[d * TOKC:(d + 1) * TOKC] += r["out"].T
    return acc.reshape(B, S, H)


# revision 17
# speedup vs baseline: 1.0006x; 1.0006x over previous
"""Trainium2 Bass kernel for nn_MixedSparseGatedMLP (LoRA-augmented gated MLP).

Math (reference):
    y1 = x @ Wg + (x @ Ag) @ Bg
    y2 = x @ Wu + (x @ Au) @ Bu
    x3 = relu(y1) * y2
    y3 = x3 @ Wd + (x3 @ Ad) @ Bd

Strategy:
  - Fold the rank-16 LoRA factors into the dense weights on the host
    (exact fp32 algebra): Wg_eff = Wg + Ag@Bg, etc.  The device kernel is
    then a plain gated MLP with three dense matmuls.
  - TP2 x DP4 sharding: I = 11008 = 86*128 chunks split into 2
    tensor-parallel groups of 43 chunks (no padding, unlike an 8-way
    split which needs 88), and tokens split 4 ways (1024 per core, as
    2 blocks of 512).  Core c: group g=c//4, token range d=c%4.
    Each core emits a partial out^T [H, 1024]; the host adds the two
    group partials per token range (gather/unshard step).
  - Down projection runs in out^T orientation: stationary = Wd slice
    [128i x 128h], moving = x3 [128i x 512t], one PSUM tile per
    128-row h-block, 43-deep accumulation.  Wd streams through SBUF
    exactly once per block (it is far too big to keep resident).
  - bf16 operands, fp32 PSUM accumulation, fp32 partial outputs.
  - All DRAM layouts are pre-tiled on the host so every DMA is a linear
    copy into the exact SBUF layout the matmuls need.
"""

import os
import sys

for _p in ("/opt/trn_rl_repo", "/root/.axon_site/_ro/trn_rl_repo"):
    if os.path.isdir(_p) and _p not in sys.path:
        sys.path.append(_p)

import numpy as np
import ml_dtypes

# Problem shapes (hardcoded per contract)
B, S, H, I, R = 2, 2048, 4096, 11008, 16
NTOK = B * S              # 4096 tokens
NCORES = 8
TPG = 2                   # tensor-parallel groups over I
DPG = 4                   # data-parallel groups over tokens
CI = I // 128             # 86 i-chunks total
C = CI // TPG             # 43 i-chunks per core
IS = C * 128              # 5504 intermediate columns per core
K = H // 128              # 32 h-chunks (gate/up contraction)
TB = 512                  # token block
TOKC = NTOK // DPG        # 1024 tokens per core
NB = TOKC // TB           # 2 token blocks per core
NHB = H // 128            # 32 output h-blocks (down phase)
KG = 4                    # x k-groups per block (startup split)
KS = K // KG

BF16 = ml_dtypes.bfloat16

# set by test.py for profiling; harness path leaves these as-is
TRACE = False
LAST_EXEC_TIME_NS = None
LAST_RESULTS = None


def _build_nc():
    import concourse.bacc as bacc
    import concourse.mybir as mybir
    import concourse.tile as tile

    bf16 = mybir.dt.bfloat16
    f32 = mybir.dt.float32

    nc = bacc.Bacc("TRN2", target_bir_lowering=False, debug=False)

    # DRAM parameters (host pre-tiled layouts; see kernel() for the math)
    x = nc.declare_dram_parameter("x", [NB, 128, K * TB], bf16, isOutput=False)
    wg = nc.declare_dram_parameter("wg", [C, 128, K * 128], bf16, isOutput=False)
    wu = nc.declare_dram_parameter("wu", [C, 128, K * 128], bf16, isOutput=False)
    wd = nc.declare_dram_parameter("wd", [NHB, 128, C * 128], bf16, isOutput=False)
    out = nc.declare_dram_parameter("out", [H, TOKC], f32, isOutput=True)

    SC = 3   # startup chunks processed k-interleaved (see below)
    HK = K // 2

    with tile.TileContext(nc) as tc:
        with tc.tile_pool(name="xp", bufs=1) as xp, \
             tc.tile_pool(name="wp", bufs=4) as wp, \
             tc.tile_pool(name="wsp", bufs=1) as wsp, \
             tc.tile_pool(name="wdp", bufs=3) as wdp, \
             tc.tile_pool(name="x3p", bufs=1) as x3p, \
             tc.tile_pool(name="rp", bufs=2) as rp, \
             tc.tile_pool(name="op", bufs=4) as op, \
             tc.tile_pool(name="pgp", bufs=3, space="PSUM") as pgp, \
             tc.tile_pool(name="pup", bufs=3, space="PSUM") as pup, \
             tc.tile_pool(name="pdp", bufs=2, space="PSUM") as pdp:

            # PE warmup: dependency-free matmuls run during the initial DMA
            # wait and lift the HAM clock gate to 8/8 before real work.
            # The interleaved startup below begins at ~12us, so only a few
            # are needed to bridge from sequencer start (~9us).
            warm_in = rp.tile([128, TB], bf16, tag="warm")
            nc.any.memset(warm_in, 0.0)
            warm_ps = pdp.tile([128, TB], f32, tag="pd")
            for _ in range(8):
                nc.tensor.matmul(warm_ps, warm_in[:, 0:128], warm_in,
                                 start=True, stop=True)

            def w_tile(src, m, eng=None):
                # monolithic 1MB weight DMAs: large transfers fan out over
                # more DMA engines and sustain ~2x the per-queue bandwidth
                # of 256KB tiles
                t = wp.tile([128, K * 128], bf16, tag="w")
                (eng or nc.sync).dma_start(t, src[m])
                return t

            def wd_tile(n, eng=None):
                t = wdp.tile([128, C * 128], bf16, tag="wdn")
                (eng or nc.sync).dma_start(t, wd[n])
                return t

            # prefetched tiles: (b, m) -> (wg_tile, wu_tile); (b, n) -> wd tile
            wpref = {}
            wdpref = {}
            xbg_next = {}

            for b in range(NB):
                # x block, split into KG tiles: [128 h-in-chunk, (k, t)] bf16
                xbg = []
                if b == 0:
                    # Startup ramp is HBM-bound with no previous block to
                    # hide under, and a single chunk's 64-matmul burst
                    # consumes x at ~590GB/s vs ~300GB/s DMA delivery.
                    # So the first SC=3 chunks run k-group-interleaved
                    # (6 open PSUM groups), dropping consumption to
                    # ~245GB/s; weights for them stream as 512KB
                    # half-tiles, everything issued in need-time order
                    # across both DGE paths.
                    xbg = [None] * KG

                    def xdma(gi, eng):
                        t = xp.tile([128, KS * TB], bf16, tag=f"xb{gi}")
                        eng.dma_start(t, x[0][:, gi * KS * TB:(gi + 1) * KS * TB])
                        xbg[gi] = t

                    wq = {}

                    def wqdma(c, w, h, eng):
                        src = wg if w == 'g' else wu
                        t = wsp.tile([128, HK * 128], bf16, tag=f"ws{c}{w}{h}")
                        eng.dma_start(t, src[c][:, h * HK * 128:(h + 1) * HK * 128])
                        wq[(c, w, h)] = t

                    xdma(0, nc.sync)
                    wqdma(0, 'g', 0, nc.gpsimd)
                    wqdma(0, 'u', 0, nc.sync)
                    wqdma(1, 'g', 0, nc.gpsimd)
                    wqdma(1, 'u', 0, nc.sync)
                    wqdma(2, 'g', 0, nc.gpsimd)
                    wqdma(2, 'u', 0, nc.gpsimd)
                    xdma(1, nc.sync)
                    xdma(2, nc.sync)
                    wqdma(0, 'g', 1, nc.gpsimd)
                    wqdma(0, 'u', 1, nc.sync)
                    wqdma(1, 'g', 1, nc.gpsimd)
                    wqdma(1, 'u', 1, nc.gpsimd)
                    wqdma(2, 'g', 1, nc.gpsimd)
                    xdma(3, nc.sync)
                    wqdma(2, 'u', 1, nc.gpsimd)
                else:
                    # prefetched during the previous down phase (SWDGE)
                    xbg = [xbg_next.pop(gi) for gi in range(KG)]

                # x3^T for this block: [128 i-in-chunk, (c, t)] bf16
                x3 = x3p.tile([128, C * TB], bf16, tag="x3")

                if b == 0:
                    # ---- interleaved startup: chunks 0..SC-1 ----
                    psg = [pgp.tile([128, TB], f32, tag="pg", name=f"psg{c}")
                           for c in range(SC)]
                    psu = [pup.tile([128, TB], f32, tag="pu", name=f"psu{c}")
                           for c in range(SC)]
                    for kg in range(KG):
                        h, hk = divmod(kg, 2)
                        for c in range(SC):
                            for w in ('g', 'u'):
                                ps = psg[c] if w == 'g' else psu[c]
                                t = wq[(c, w, h)]
                                for kk in range(KS):
                                    nc.tensor.matmul(
                                        ps,
                                        t[:, (hk * KS + kk) * 128:(hk * KS + kk + 1) * 128],
                                        xbg[kg][:, kk * TB:(kk + 1) * TB],
                                        start=(kg == 0 and kk == 0),
                                        stop=(kg == KG - 1 and kk == KS - 1),
                                    )
                            if kg == KG - 1:
                                r = rp.tile([128, TB], bf16, tag="r")
                                nc.scalar.activation(
                                    r, psg[c], mybir.ActivationFunctionType.Relu)
                                nc.vector.tensor_mul(
                                    x3[:, c * TB:(c + 1) * TB], r, psu[c])

                # ---- gate / up projections + gating, per i-chunk m ----
                for m in range(SC if b == 0 else 0, C):
                    if (b, m) in wpref:
                        wgt, wut = wpref.pop((b, m))
                    else:
                        # block 0: split the weight stream across both DGE
                        # paths (no down phase to hide under); later blocks
                        # stream both on HWDGE (146 GB/s, proven) while
                        # SWDGE stays free for prefetches.
                        wgt = w_tile(wg, m)
                        wut = w_tile(wu, m, nc.gpsimd if b == 0 else None)

                    g = pgp.tile([128, TB], f32, tag="pg")
                    u = pup.tile([128, TB], f32, tag="pu")
                    for k in range(K):
                        gi, kk = divmod(k, KS)
                        nc.tensor.matmul(
                            g,
                            wgt[:, k * 128:(k + 1) * 128],
                            xbg[gi][:, kk * TB:(kk + 1) * TB],
                            start=(k == 0), stop=(k == K - 1),
                        )
                    for k in range(K):
                        gi, kk = divmod(k, KS)
                        nc.tensor.matmul(
                            u,
                            wut[:, k * 128:(k + 1) * 128],
                            xbg[gi][:, kk * TB:(kk + 1) * TB],
                            start=(k == 0), stop=(k == K - 1),
                        )
                    # x3 = relu(g) * u ; DVE may read only one PSUM input,
                    # so relu lands in SBUF via ACT first.
                    r = rp.tile([128, TB], bf16, tag="r")
                    nc.scalar.activation(r, g, mybir.ActivationFunctionType.Relu)
                    nc.vector.tensor_mul(x3[:, m * TB:(m + 1) * TB], r, u)

                    if b == 0:
                        # wd prefetch for block 0's down phase: both DGE
                        # queues are lockstepped with the weight stream by
                        # wp backpressure, so insert the first wd tiles into
                        # the streams near the tail — they land ~2 chunks
                        # (~27us) before the down phase needs them.
                        if m == C - 3:
                            wdpref[(0, 0)] = wd_tile(0, nc.sync)
                        elif m == C - 2:
                            wdpref[(0, 1)] = wd_tile(1, nc.gpsimd)
                        elif m == C - 1:
                            wdpref[(0, 2)] = wd_tile(2, nc.sync)
                    elif m == 0:
                        # SWDGE is idle during later gate phases: pull the
                        # whole wd prefetch window up front.
                        for n in range(3):
                            wdpref[(b, n)] = wd_tile(n, nc.gpsimd)

                # ---- down projection, out^T orientation ----
                # psum [128 h, TB tok] accumulates over all 43 i-chunks;
                # wd streams h-block by h-block on HWDGE (JIT via wdp
                # backpressure), stores ride SWDGE.
                for n in range(NHB):
                    wdt = wdpref.pop((b, n), None)
                    if wdt is None:
                        wdt = wd_tile(n)
                    d = pdp.tile([128, TB], f32, tag="pd")
                    for c in range(C):
                        nc.tensor.matmul(
                            d,
                            wdt[:, c * 128:(c + 1) * 128],
                            x3[:, c * TB:(c + 1) * TB],
                            start=(c == 0), stop=(c == C - 1),
                        )
                    o = op.tile([128, TB], f32, tag="o")
                    nc.scalar.copy(o, d)
                    nc.gpsimd.dma_start(
                        out[n * 128:(n + 1) * 128, b * TB:(b + 1) * TB], o
                    )

                    if b < NB - 1:
                        # prefetch the next block's x and first gate/up
                        # weights on SWDGE while HWDGE streams wd
                        if n in (2, 4, 6, 8):
                            gi = (n - 2) // 2
                            t = xp.tile([128, KS * TB], bf16, tag=f"xb{gi}")
                            nc.gpsimd.dma_start(
                                t, x[b + 1][:, gi * KS * TB:(gi + 1) * KS * TB]
                            )
                            xbg_next[gi] = t
                        elif n == 26:
                            wgt1 = w_tile(wg, 0, nc.gpsimd)
                        elif n == 28:
                            wut1 = w_tile(wu, 0, nc.gpsimd)
                            wpref[(b + 1, 0)] = (wgt1, wut1)

    nc.compile()
    return nc


def _prep_inputs(x1, w_gate, w_gate_lora_a, w_gate_lora_b,
                 w_up, w_up_lora_a, w_up_lora_b,
                 w_down, w_down_lora_a, w_down_lora_b):
    """Fold LoRA, shard TP2xDP4, and pre-tile DRAM layouts."""
    f32 = np.float32
    x1 = np.asarray(x1, f32)
    wg_eff = np.asarray(w_gate, f32) + np.asarray(w_gate_lora_a, f32) @ np.asarray(w_gate_lora_b, f32)
    wu_eff = np.asarray(w_up, f32) + np.asarray(w_up_lora_a, f32) @ np.asarray(w_up_lora_b, f32)
    wd_eff = np.asarray(w_down, f32) + np.asarray(w_down_lora_a, f32) @ np.asarray(w_down_lora_b, f32)

    x2d = x1.reshape(NTOK, H)

    # per-group weight tilings
    gmaps = []
    for g in range(TPG):
        sl = slice(g * IS, (g + 1) * IS)
        # wg tile layout: [m, p, k*128+j] = wg_eff[k*128+p, g*IS + m*128 + j]
        wgc = np.ascontiguousarray(
            wg_eff[:, sl].reshape(K, 128, C, 128).transpose(2, 1, 0, 3)
        ).astype(BF16).reshape(C, 128, K * 128)
        wuc = np.ascontiguousarray(
            wu_eff[:, sl].reshape(K, 128, C, 128).transpose(2, 1, 0, 3)
        ).astype(BF16).reshape(C, 128, K * 128)
        # wd tile layout: [n, p, c*128+h] = wd_eff[g*IS + c*128 + p, n*128+h]
        wdc = np.ascontiguousarray(
            wd_eff[sl, :].reshape(C, 128, NHB, 128).transpose(2, 1, 0, 3)
        ).astype(BF16).reshape(NHB, 128, C * 128)
        gmaps.append((wgc, wuc, wdc))

    # per-DP-slice x tilings: x_t[b, p, k*TB+t] = x2d[d*TOKC + b*TB + t, k*128+p]
    xmaps = []
    for d in range(DPG):
        xs = x2d[d * TOKC:(d + 1) * TOKC]
        xt = np.ascontiguousarray(
            xs.reshape(NB, TB, K, 128).transpose(0, 3, 2, 1)
        ).astype(BF16).reshape(NB, 128, K * TB)
        xmaps.append(xt)

    in_maps = []
    for ci in range(NCORES):
        g, d = divmod(ci, DPG)
        wgc, wuc, wdc = gmaps[g]
        in_maps.append({"x": xmaps[d], "wg": wgc, "wu": wuc, "wd": wdc})
    return in_maps


def _emulate(in_maps):
    """Numpy emulation of the device math (bf16 operands, fp32 accum),
    reconstructing operands from the tiled layouts to validate them."""
    f32 = np.float32
    acc = np.zeros((NTOK, H), f32)
    for ci, m in enumerate(in_maps):
        g, d = divmod(ci, DPG)
        xt = m["x"].reshape(NB, 128, K, TB)
        xs = xt.transpose(0, 3, 2, 1).reshape(TOKC, H).astype(f32)
        wgc = m["wg"].reshape(C, 128, K, 128)
        wg2 = wgc.transpose(2, 1, 0, 3).reshape(H, IS).astype(f32)
        wuc = m["wu"].reshape(C, 128, K, 128)
        wu2 = wuc.transpose(2, 1, 0, 3).reshape(H, IS).astype(f32)
        wdc = m["wd"].reshape(NHB, 128, C, 128)
        wd2 = wdc.transpose(2, 1, 0, 3).reshape(IS, H).astype(f32)
        y1 = xs @ wg2
        y2 = xs @ wu2
        r = np.maximum(y1, 0).astype(BF16).astype(f32)
        x3 = (r * y2).astype(BF16).astype(f32)
        acc[d * TOKC:(d + 1) * TOKC] += x3 @ wd2
    return acc.reshape(B, S, H)


def kernel(**inputs):
    global LAST_EXEC_TIME_NS, LAST_RESULTS
    in_maps = _prep_inputs(**inputs)

    if os.environ.get("KERNEL_EMULATE"):
        return _emulate(in_maps)

    from concourse.bass_utils import run_bass_kernel_spmd

    nc = _build_nc()
    res = run_bass_kernel_spmd(nc, in_maps, list(range(NCORES)), trace=TRACE)
    LAST_EXEC_TIME_NS = res.exec_time_ns
    LAST_RESULTS = res

    acc = np.zeros((NTOK, H), np.float32)
    for ci, r in enumerate(res.results):
        g, d = divmod(ci, DPG)
        acc[d * TOKC:(d + 1) * TOKC] += r["out"].T
    return acc.reshape(B, S, H)
